# revision 13
# baseline (speedup 1.0000x reference)
"""Self-contained Trainium2 Bass kernel for a 2-layer GAT (PyG GATConv semantics).

Strategy (8 NeuronCores, SPMD):
  - dst-node partitioning: core c owns global nodes [c*SH, (c+1)*SH) (last core
    padded). Within a core, nodes are sorted by in-degree and processed in
    batches of 128 (one node per SBUF partition); per-batch slot counts are
    padded to a cross-core-uniform schedule Dm[b].
  - per layer: node-sharded matmul produces the augmented node table
    [h | alpha_src | alpha_dst] in one PE matmul per 128-node tile (attention
    projections folded into the weight matrix on host), AllGather of the
    528-byte-row node table, then an edge phase that indirect-DMA-gathers
    h[src]|alpha_src[src] rows and reduces sum(w * h[src]) per dst node with
    strided vector reduces.
  - softmax is computed without segment_max (the shift cancels exactly in
    num/den; logits here are far below exp overflow).
  - exp(leaky_relu(x, 0.2)) is computed as max(exp(x), exp(0.2 x)).
  - degree-0 pad nodes / pad slots are neutralized by a -1e30 alpha_src pad row
    (global row id N) and den += 1e-30.
"""

import numpy as np
from contextlib import ExitStack


# ---------------------------------------------------------------- config

class Cfg:
    def __init__(self, N, E, SH):
        self.N = N                  # real nodes
        self.E = E                  # edges before self-loops
        self.SH = SH                # shard rows per core (multiple of 128)
        self.NCORES = 8
        self.NPAD = 8 * SH
        self.PB = 128
        self.NB = SH // 128
        self.F = 128                # feature dim (in/out of both layers)
        self.H1, self.C1 = 4, 32    # layer-1 heads
        assert 7 * SH <= N < 8 * SH
        assert SH % 128 == 0


FULL = Cfg(N=100000, E=1600000, SH=12544)


# ---------------------------------------------------------------- host prep

def host_prep(cfg, edge_index):
    """Degree-sort nodes per core, build the uniform batch schedule and the
    per-core gather-offset arrays (pi-space row ids)."""
    N, SH, PB, NB, NC = cfg.N, cfg.SH, cfg.PB, cfg.NB, cfg.NCORES
    src = np.concatenate([edge_index[0], np.arange(N, dtype=np.int64)])
    dst = np.concatenate([edge_index[1], np.arange(N, dtype=np.int64)])
    core_of = dst // SH

    perms = np.empty((NC, SH), np.int64)
    invs = np.empty((NC, SH), np.int64)
    sorted_degs = np.empty((NC, SH), np.int64)
    edges = []
    for c in range(NC):
        m = core_of == c
        d_loc = dst[m] - c * SH
        deg = np.bincount(d_loc, minlength=SH)
        perm = np.argsort(-deg, kind="stable")
        perms[c] = perm
        invs[c, perm] = np.arange(SH)
        sorted_degs[c] = deg[perm]
        edges.append((d_loc, src[m]))

    Dm = sorted_degs[:, ::PB][:, :NB].max(axis=0).astype(np.int64)  # [NB]
    offs = np.zeros(NB + 1, np.int64)
    np.cumsum(Dm, out=offs[1:])
    S = int(offs[-1])

    offsets = np.full((NC, PB, S), cfg.N, dtype=np.int32)  # pad row = pi-id N
    for c in range(NC):
        d_loc, s_glb = edges[c]
        pos = invs[c, d_loc]
        order = np.argsort(pos, kind="stable")
        pos_s = pos[order]
        src_s = s_glb[order]
        first = np.searchsorted(pos_s, pos_s)
        slot = np.arange(len(pos_s)) - first
        c2 = src_s // SH
        src_pi = c2 * SH + invs[c2, src_s - c2 * SH]
        offsets[c, pos_s % PB, offs[pos_s // PB] + slot] = src_pi

    return perms, Dm, offs[:-1], S, offsets


def make_wcats(cfg, W1, a_src1, a_dst1, W2, a_src2, a_dst2):
    F, H1, C1 = cfg.F, cfg.H1, cfg.C1
    W1T = np.ascontiguousarray(W1.T, dtype=np.float32)        # [F, H1*C1]
    Bs1 = np.einsum("hck,hc->kh", W1.reshape(H1, C1, F), a_src1)
    Bd1 = np.einsum("hck,hc->kh", W1.reshape(H1, C1, F), a_dst1)
    wcat1 = np.concatenate([W1T, Bs1, Bd1], 1).astype(np.float32)  # [128,136]
    W2T = np.ascontiguousarray(W2.T, dtype=np.float32)
    Bs2 = (W2.T @ a_src2[0])[:, None]
    Bd2 = (W2.T @ a_dst2[0])[:, None]
    wcat2 = np.concatenate([W2T, Bs2, Bd2], 1).astype(np.float32)  # [128,130]
    return wcat1, wcat2


def make_core_inputs(cfg, x, perms, offsets, wcat1, wcat2, b1):
    """Per-core input dicts."""
    N, SH, NC = cfg.N, cfg.SH, cfg.NCORES
    maps = []
    b1_bcast = np.broadcast_to(b1.astype(np.float32), (128, 128)).copy()
    for c in range(NC):
        base = c * SH
        cnt = min(SH, N - base)
        perm = perms[c]
        valid = perm < cnt
        xs = np.zeros((SH, cfg.F), np.float32)
        xs[valid] = x[base + perm[valid]]
        p1 = np.zeros((SH, 4), np.float32)
        p1[~valid] = -1e30
        p2 = np.zeros((SH, 1), np.float32)
        p2[~valid] = -1e30
        maps.append({
            "x_shard": xs,
            "offsets": np.ascontiguousarray(offsets[c]),
            "wcat1": wcat1, "wcat2": wcat2,
            "patch1": p1, "patch2": p2,
            "bias1": b1_bcast,
        })
    return maps


# ---------------------------------------------------------------- bass program

def split_multi_waits(nc):
    """This walrus build only accepts ONE embedded semaphore wait per
    instruction (setupSyncWait: 'Too many sync wait commands').  Move all but
    the last wait of any multi-wait instruction onto same-engine NoOps
    inserted immediately before it — semantically identical (the engine
    queue blocks on each wait in order before executing the instruction)."""
    import concourse.mybir as mybir
    import bass_rust
    n_split = 0
    for f in nc.m.functions:
        for bb in f.blocks:
            lst = bb.instructions
            i = 0
            while i < len(lst):
                inst = lst[i]
                si = inst.sync_info
                if si is not None and len(si.on_wait) > 1:
                    waits = list(si.on_wait)
                    for k, w in enumerate(waits[:-1]):
                        nop = mybir.InstNoOp(name=f"{inst.name}-w{k}", ins=[], outs=[])
                        nop.engine = inst.engine
                        nop.sync_info = bass_rust.SyncInfo(on_wait=[w], on_update=[])
                        lst.insert(i, nop)
                        i += 1
                    inst.sync_info = bass_rust.SyncInfo(
                        on_wait=[waits[-1]], on_update=list(si.on_update))
                    n_split += 1
                i += 1
    return n_split


def build_bass(cfg, Dm, offs, S, split=True, stages=4):
    import concourse.bass as bass
    import concourse.mybir as mybir
    import concourse.tile as tile
    from concourse.masks import make_identity

    fp = mybir.dt.float32
    SH, NB, NPAD = cfg.SH, cfg.NB, cfg.NPAD
    AG_GROUPS = [list(range(cfg.NCORES))]

    nc = bass.Bass()
    x_shard = nc.declare_dram_parameter("x_shard", [SH, 128], fp, isOutput=False)
    offsets_d = nc.declare_dram_parameter("offsets", [128, S], mybir.dt.int32, isOutput=False)
    wcat1_d = nc.declare_dram_parameter("wcat1", [128, 136], fp, isOutput=False)
    wcat2_d = nc.declare_dram_parameter("wcat2", [128, 130], fp, isOutput=False)
    patch1_d = nc.declare_dram_parameter("patch1", [SH, 4], fp, isOutput=False)
    patch2_d = nc.declare_dram_parameter("patch2", [SH, 1], fp, isOutput=False)
    bias1_d = nc.declare_dram_parameter("bias1", [128, 128], fp, isOutput=False)
    out_d = nc.declare_dram_parameter("out", [SH, 128], fp, isOutput=True)

    haug1_loc = nc.dram_tensor("haug1_loc", [SH, 132], fp)
    ad1_loc = nc.dram_tensor("ad1_loc", [SH, 4], fp)
    x2_loc = nc.dram_tensor("x2_loc", [SH, 128], fp)
    haug2_loc = nc.dram_tensor("haug2_loc", [SH, 129], fp)
    ad2_loc = nc.dram_tensor("ad2_loc", [SH, 1], fp)
    haug1_tab = nc.dram_tensor("haug1_tab", [NPAD, 132], fp, addr_space="Shared")
    haug2_tab = nc.dram_tensor("haug2_tab", [NPAD, 129], fp, addr_space="Shared")

    def vap(t, free_dims):
        """View of a tile with explicit free-dim [step, count] pairs."""
        a = t[tuple([slice(None)] * len(t.shape))]
        return bass.AP(tensor=a.tensor, offset=a.offset, ap=[a.ap[0]] + free_dims)

    with tile.TileContext(nc) as tc, ExitStack() as ctx:
        consts = ctx.enter_context(tc.tile_pool(name="consts", bufs=1))
        ident = consts.tile([128, 128], fp)
        make_identity(nc, ident[:])
        # PE observes the identity producer's clock once, so per-tile
        # transposes carry a single (DMA) wait — walrus's LDWEIGHTS struct
        # only tolerates one sync wait.
        warm_ps = ctx.enter_context(tc.tile_pool(name="warm_ps", bufs=2, space="PSUM"))

        def pe_sync():
            # Dummy PE matmul: absorbs cross-engine waits (Pool/SP barrier)
            # so real transposes/matmuls carry a single sync wait each.
            pe_warm = warm_ps.tile([128, 1], fp, space="PSUM", tag="pe_warm")
            nc.tensor.matmul(out=pe_warm[:], lhsT=ident[:], rhs=ident[:, 0:1],
                             start=True, stop=True)

        pe_sync()
        wc1_dma = consts.tile([128, 136], fp)
        nc.sync.dma_start(out=wc1_dma[:], in_=wcat1_d[:, :])
        wc1_sb = consts.tile([128, 136], fp)
        nc.vector.tensor_copy(out=wc1_sb[:], in_=wc1_dma[:])
        wc2_dma = consts.tile([128, 130], fp)
        nc.sync.dma_start(out=wc2_dma[:], in_=wcat2_d[:, :])
        wc2_sb = consts.tile([128, 130], fp)
        nc.vector.tensor_copy(out=wc2_sb[:], in_=wc2_dma[:])
        b1_sb = consts.tile([128, 128], fp)
        nc.sync.dma_start(out=b1_sb[:], in_=bias1_d[:, :])
        offs_sb = consts.tile([128, S], mybir.dt.int32)
        nc.sync.dma_start(out=offs_sb[:], in_=offsets_d[:, :])

        mm_x = ctx.enter_context(tc.tile_pool(name="mm_x", bufs=3))
        mm_ps = ctx.enter_context(tc.tile_pool(name="mm_ps", bufs=2, space="PSUM"))
        mm_st = ctx.enter_context(tc.tile_pool(name="mm_st", bufs=3))

        def matmul_phase(src_dram, wc_sb, ncols, patch_dram, pw, haug_dram, hw,
                         ad_dram, elu_in: bool):
            """haug rows = [h | alpha_src(+patch)] ; alpha_dst -> ad_dram."""
            for t in range(NB):
                r0 = t * 128
                x_t = mm_x.tile([128, 128], fp, tag="x_t")
                nc.sync.dma_start(out=x_t[:], in_=src_dram[r0:r0 + 128, :])
                if elu_in:
                    # x2 = elu(raw + b1):  max(z,0) + exp(min(z,0)) - 1
                    z = mm_x.tile([128, 128], fp, tag="z")
                    nc.vector.tensor_tensor(out=z[:], in0=x_t[:], in1=b1_sb[:],
                                            op=mybir.AluOpType.add)
                    nc.vector.tensor_scalar_max(x_t[:], z[:], 0.0)
                    nc.vector.tensor_scalar_min(z[:], z[:], 0.0)
                    nc.scalar.activation(z[:], z[:], mybir.ActivationFunctionType.Exp)
                    nc.vector.tensor_tensor(out=x_t[:], in0=x_t[:], in1=z[:],
                                            op=mybir.AluOpType.add)
                    nc.vector.tensor_scalar_add(x_t[:], x_t[:], -1.0)
                xt_ps = mm_ps.tile([128, 128], fp, space="PSUM", tag="xt_ps")
                nc.tensor.transpose(out=xt_ps[:], in_=x_t[:], identity=ident[:])
                xt_sb = mm_x.tile([128, 128], fp, tag="xt_sb")
                nc.vector.tensor_copy(out=xt_sb[:], in_=xt_ps[:])
                o_ps = mm_ps.tile([128, ncols], fp, space="PSUM", tag="o_ps")
                nc.tensor.matmul(out=o_ps[:], lhsT=xt_sb[:], rhs=wc_sb[:, :ncols],
                                 start=True, stop=True)
                st = mm_st.tile([128, ncols], fp, tag="st")
                nc.vector.tensor_copy(out=st[:], in_=o_ps[:])
                pt = mm_st.tile([128, pw], fp, tag="pt")
                nc.sync.dma_start(out=pt[:], in_=patch_dram[r0:r0 + 128, :])
                nc.vector.tensor_tensor(out=st[:, 128:128 + pw],
                                        in0=st[:, 128:128 + pw], in1=pt[:],
                                        op=mybir.AluOpType.add)
                nc.sync.dma_start(out=haug_dram[r0:r0 + 128, 0:hw], in_=st[:, 0:hw])
                nc.sync.dma_start(out=ad_dram[r0:r0 + 128, :],
                                  in_=st[:, 128 + pw:128 + 2 * pw])

        eg_g = ctx.enter_context(tc.tile_pool(name="eg_g", bufs=2))
        eg_w = ctx.enter_context(tc.tile_pool(name="eg_w", bufs=2))
        eg_m = ctx.enter_context(tc.tile_pool(name="eg_m", bufs=2))
        eg_s = ctx.enter_context(tc.tile_pool(name="eg_s", bufs=3))
        eg_o = ctx.enter_context(tc.tile_pool(name="eg_o", bufs=3))

        def edge_phase(haug_tab, ad_dram, H, out_dram, W):
            Cc = 128 // H
            for b in range(NB):
                d = int(Dm[b])
                o = int(offs[b])
                r0 = b * 128
                ad_t = eg_s.tile([128, H], fp, tag="ad")
                nc.sync.dma_start(out=ad_t[:], in_=ad_dram[r0:r0 + 128, :])
                G = eg_g.tile([128, d, W], fp, tag="G")
                # HW indirect DMA only honors the one-offset-per-partition
                # form (one descriptor per partition): gather one slot-column
                # (128 rows x W) per call.
                for s in range(d):
                    nc.gpsimd.indirect_dma_start(
                        out=G[:, s, :], out_offset=None, in_=haug_tab[:, :],
                        in_offset=bass.IndirectOffsetOnAxis(
                            ap=offs_sb[:, o + s:o + s + 1], axis=0))
                # logit = alpha_src[src] + alpha_dst[dst]
                logit = eg_w.tile([128, d, H], fp, tag="logit")
                if H == 1:
                    nc.vector.tensor_scalar(
                        out=logit[:, :, 0:1], in0=G[:, :, 128:129],
                        scalar1=ad_t[:, 0:1], scalar2=None,
                        op0=mybir.AluOpType.add)
                else:
                    nc.vector.tensor_tensor(
                        out=logit[:, :, :], in0=G[:, :, 128:128 + H],
                        in1=vap(ad_t, [[0, d], [1, H]]), op=mybir.AluOpType.add)
                # w = exp(leaky_relu(logit)) = max(exp(logit), exp(0.2 logit))
                e1 = eg_w.tile([128, d, H], fp, tag="e1")
                fl = lambda t: t[:].rearrange("p k h -> p (k h)")
                nc.scalar.activation(fl(e1), fl(logit), mybir.ActivationFunctionType.Exp)
                wt = eg_w.tile([128, d, H], fp, tag="wt")
                nc.scalar.activation(fl(wt), fl(logit), mybir.ActivationFunctionType.Exp,
                                     scale=0.2)
                nc.vector.tensor_tensor(out=wt[:, :, :], in0=wt[:, :, :],
                                        in1=e1[:, :, :], op=mybir.AluOpType.max)
                # den/recip
                den = eg_s.tile([128, H], fp, tag="den")
                nc.vector.tensor_reduce(out=den[:, :], in_=vap(wt, [[1, H], [H, d]]),
                                        axis=mybir.AxisListType.X, op=mybir.AluOpType.add)
                nc.vector.tensor_scalar_add(den[:, :], den[:, :], 1e-30)
                rec = eg_s.tile([128, H], fp, tag="rec")
                nc.vector.reciprocal(rec[:, :], den[:, :])
                # msg = w (bcast over Cc) * h[src]
                msg = eg_m.tile([128, d, 128], fp, tag="msg")
                nc.vector.tensor_tensor(
                    out=vap(msg, [[128, d], [Cc, H], [1, Cc]]),
                    in0=vap(G, [[W, d], [Cc, H], [1, Cc]]),
                    in1=vap(wt, [[H, d], [1, H], [0, Cc]]),
                    op=mybir.AluOpType.mult)
                num = eg_o.tile([128, 128], fp, tag="num")
                nc.vector.tensor_reduce(out=num[:, :], in_=vap(msg, [[1, 128], [128, d]]),
                                        axis=mybir.AxisListType.X, op=mybir.AluOpType.add)
                outt = eg_o.tile([128, 128], fp, tag="outt")
                if H == 1:
                    nc.vector.tensor_scalar_mul(outt[:, :], num[:, :], rec[:, 0:1])
                else:
                    nc.vector.tensor_tensor(
                        out=vap(outt, [[Cc, H], [1, Cc]]),
                        in0=vap(num, [[Cc, H], [1, Cc]]),
                        in1=vap(rec, [[1, H], [0, Cc]]),
                        op=mybir.AluOpType.mult)
                nc.sync.dma_start(out=out_dram[r0:r0 + 128, :], in_=outt[:, :])

        # ---------------- layer 1 ----------------
        matmul_phase(x_shard, wc1_sb, 136, patch1_d, 4, haug1_loc, 132, ad1_loc,
                     elu_in=False)
        tc.strict_bb_all_engine_barrier()
        nc.gpsimd.collective_compute(
            "AllGather", mybir.AluOpType.bypass,
            ins=[haug1_loc[:, :]], outs=[haug1_tab[:, :]],
            replica_groups=AG_GROUPS)
        tc.strict_bb_all_engine_barrier()
        if stages >= 2:
            edge_phase(haug1_tab, ad1_loc, cfg.H1, x2_loc, 132)
            tc.strict_bb_all_engine_barrier()
            pe_sync()
        if stages >= 3:
            # ---------------- layer 2 ----------------
            matmul_phase(x2_loc, wc2_sb, 130, patch2_d, 1, haug2_loc, 129, ad2_loc,
                         elu_in=True)
            tc.strict_bb_all_engine_barrier()
            nc.gpsimd.collective_compute(
                "AllGather", mybir.AluOpType.bypass,
                ins=[haug2_loc[:, :]], outs=[haug2_tab[:, :]],
                replica_groups=AG_GROUPS)
            tc.strict_bb_all_engine_barrier()
        if stages >= 4:
            edge_phase(haug2_tab, ad2_loc, 1, out_d, 129)

    if split:
        split_multi_waits(nc)
    return nc


# ---------------------------------------------------------------- entry point

def run(cfg, inputs, trace=False):
    from concourse.bass_utils import run_bass_kernel_spmd

    x = np.asarray(inputs["x"], dtype=np.float32)
    edge_index = np.asarray(inputs["edge_index"]).astype(np.int64)
    perms, Dm, offs, S, offsets = host_prep(cfg, edge_index)
    wcat1, wcat2 = make_wcats(
        cfg, np.asarray(inputs["W1"], np.float32), np.asarray(inputs["a_src1"], np.float32),
        np.asarray(inputs["a_dst1"], np.float32), np.asarray(inputs["W2"], np.float32),
        np.asarray(inputs["a_src2"], np.float32), np.asarray(inputs["a_dst2"], np.float32))
    in_maps = make_core_inputs(cfg, x, perms, offsets, wcat1, wcat2,
                               np.asarray(inputs["b1"], np.float32))
    nc = build_bass(cfg, Dm, offs, S)
    res = run_bass_kernel_spmd(nc, in_maps, list(range(cfg.NCORES)), trace=trace)

    out = np.zeros((cfg.N, 128), np.float32)
    for c in range(cfg.NCORES):
        base = c * cfg.SH
        cnt = min(cfg.SH, cfg.N - base)
        perm = perms[c]
        valid = perm < cnt
        shard = res.results[c]["out"]
        out[base + perm[valid]] = shard[valid]
    out += np.asarray(inputs["b2"], np.float32)[None, :]
    return out, res


def kernel(**inputs) -> np.ndarray:
    out, _ = run(FULL, inputs, trace=False)
    return out


# revision 15
# speedup vs baseline: 6.3618x; 6.3618x over previous
"""Self-contained Trainium2 Bass kernel for a 2-layer GAT (PyG GATConv semantics).

Strategy (8 NeuronCores, SPMD):
  - dst-node partitioning: core c owns global nodes [c*SH, (c+1)*SH) (last core
    padded). Within a core, nodes are sorted by in-degree and processed in
    batches of 128 (one node per SBUF partition); per-batch slot counts are
    padded to a cross-core-uniform schedule Dm[b].
  - per layer: node-sharded matmul produces the augmented node table
    [h | alpha_src | alpha_dst] in one PE matmul per 128-node tile (attention
    projections folded into the weight matrix on host), AllGather of the
    528-byte-row node table, then an edge phase that indirect-DMA-gathers
    h[src]|alpha_src[src] rows and reduces sum(w * h[src]) per dst node with
    strided vector reduces.
  - softmax is computed without segment_max (the shift cancels exactly in
    num/den; logits here are far below exp overflow).
  - exp(leaky_relu(x, 0.2)) is computed as max(exp(x), exp(0.2 x)).
  - degree-0 pad nodes / pad slots are neutralized by a -1e30 alpha_src pad row
    (global row id N) and den += 1e-30.
"""

import numpy as np
from contextlib import ExitStack


# ---------------------------------------------------------------- config

class Cfg:
    def __init__(self, N, E, SH):
        self.N = N                  # real nodes
        self.E = E                  # edges before self-loops
        self.SH = SH                # shard rows per core (multiple of 128)
        self.NCORES = 8
        self.NPAD = 8 * SH
        self.PB = 128
        self.NB = SH // 128
        self.F = 128                # feature dim (in/out of both layers)
        self.H1, self.C1 = 4, 32    # layer-1 heads
        assert 7 * SH <= N < 8 * SH
        assert SH % 128 == 0


FULL = Cfg(N=100000, E=1600000, SH=12544)


# ---------------------------------------------------------------- host prep

def host_prep(cfg, edge_index):
    """Degree-sort nodes per core, build the uniform batch schedule and the
    per-core gather-offset arrays (pi-space row ids)."""
    N, SH, PB, NB, NC = cfg.N, cfg.SH, cfg.PB, cfg.NB, cfg.NCORES
    src = np.concatenate([edge_index[0], np.arange(N, dtype=np.int64)])
    dst = np.concatenate([edge_index[1], np.arange(N, dtype=np.int64)])
    core_of = dst // SH

    perms = np.empty((NC, SH), np.int64)
    invs = np.empty((NC, SH), np.int64)
    sorted_degs = np.empty((NC, SH), np.int64)
    edges = []
    for c in range(NC):
        m = core_of == c
        d_loc = dst[m] - c * SH
        deg = np.bincount(d_loc, minlength=SH)
        perm = np.argsort(-deg, kind="stable")
        perms[c] = perm
        invs[c, perm] = np.arange(SH)
        sorted_degs[c] = deg[perm]
        edges.append((d_loc, src[m]))

    Dm = sorted_degs[:, ::PB][:, :NB].max(axis=0).astype(np.int64)  # [NB]
    offs = np.zeros(NB + 1, np.int64)
    np.cumsum(Dm, out=offs[1:])
    S = int(offs[-1])

    offsets = np.full((NC, PB, S), cfg.N, dtype=np.int32)  # pad row = pi-id N
    for c in range(NC):
        d_loc, s_glb = edges[c]
        pos = invs[c, d_loc]
        order = np.argsort(pos, kind="stable")
        pos_s = pos[order]
        src_s = s_glb[order]
        first = np.searchsorted(pos_s, pos_s)
        slot = np.arange(len(pos_s)) - first
        c2 = src_s // SH
        src_pi = c2 * SH + invs[c2, src_s - c2 * SH]
        offsets[c, pos_s % PB, offs[pos_s // PB] + slot] = src_pi

    return perms, Dm, offs[:-1], S, offsets


def make_wcats(cfg, W1, a_src1, a_dst1, W2, a_src2, a_dst2):
    F, H1, C1 = cfg.F, cfg.H1, cfg.C1
    W1T = np.ascontiguousarray(W1.T, dtype=np.float32)        # [F, H1*C1]
    Bs1 = np.einsum("hck,hc->kh", W1.reshape(H1, C1, F), a_src1)
    Bd1 = np.einsum("hck,hc->kh", W1.reshape(H1, C1, F), a_dst1)
    wcat1 = np.concatenate([W1T, Bs1, Bd1], 1).astype(np.float32)  # [128,136]
    W2T = np.ascontiguousarray(W2.T, dtype=np.float32)
    Bs2 = (W2.T @ a_src2[0])[:, None]
    Bd2 = (W2.T @ a_dst2[0])[:, None]
    wcat2 = np.concatenate([W2T, Bs2, Bd2], 1).astype(np.float32)  # [128,130]
    return wcat1, wcat2


def make_core_inputs(cfg, x, perms, offsets, wcat1, wcat2, b1):
    """Per-core input dicts."""
    N, SH, NC = cfg.N, cfg.SH, cfg.NCORES
    maps = []
    b1_bcast = np.broadcast_to(b1.astype(np.float32), (128, 128)).copy()
    for c in range(NC):
        base = c * SH
        cnt = min(SH, N - base)
        perm = perms[c]
        valid = perm < cnt
        xs = np.zeros((SH, cfg.F), np.float32)
        xs[valid] = x[base + perm[valid]]
        p1 = np.zeros((SH, 4), np.float32)
        p1[~valid] = -1e30
        p2 = np.zeros((SH, 1), np.float32)
        p2[~valid] = -1e30
        maps.append({
            "x_shard": xs,
            "offsets": np.ascontiguousarray(offsets[c]),
            "wcat1": wcat1, "wcat2": wcat2,
            "patch1": p1, "patch2": p2,
            "bias1": b1_bcast,
        })
    return maps


# ---------------------------------------------------------------- bass program

def split_multi_waits(nc):
    """This walrus build only accepts ONE embedded semaphore wait per
    instruction (setupSyncWait: 'Too many sync wait commands').  Move all but
    the last wait of any multi-wait instruction onto same-engine NoOps
    inserted immediately before it — semantically identical (the engine
    queue blocks on each wait in order before executing the instruction)."""
    import concourse.mybir as mybir
    import bass_rust
    n_split = 0
    for f in nc.m.functions:
        for bb in f.blocks:
            lst = bb.instructions
            i = 0
            while i < len(lst):
                inst = lst[i]
                si = inst.sync_info
                if si is not None and len(si.on_wait) > 1:
                    waits = list(si.on_wait)
                    for k, w in enumerate(waits[:-1]):
                        nop = mybir.InstNoOp(name=f"{inst.name}-w{k}", ins=[], outs=[])
                        nop.engine = inst.engine
                        nop.sync_info = bass_rust.SyncInfo(on_wait=[w], on_update=[])
                        lst.insert(i, nop)
                        i += 1
                    inst.sync_info = bass_rust.SyncInfo(
                        on_wait=[waits[-1]], on_update=list(si.on_update))
                    n_split += 1
                i += 1
    return n_split


def build_bass(cfg, Dm, offs, S, split=True, stages=4, reps=1):
    import concourse.bass as bass
    import concourse.mybir as mybir
    import concourse.tile as tile
    from concourse.masks import make_identity

    fp = mybir.dt.float32
    SH, NB, NPAD = cfg.SH, cfg.NB, cfg.NPAD
    AG_GROUPS = [list(range(cfg.NCORES))]

    nc = bass.Bass()
    x_shard = nc.declare_dram_parameter("x_shard", [SH, 128], fp, isOutput=False)
    offsets_d = nc.declare_dram_parameter("offsets", [128, S], mybir.dt.int32, isOutput=False)
    wcat1_d = nc.declare_dram_parameter("wcat1", [128, 136], fp, isOutput=False)
    wcat2_d = nc.declare_dram_parameter("wcat2", [128, 130], fp, isOutput=False)
    patch1_d = nc.declare_dram_parameter("patch1", [SH, 4], fp, isOutput=False)
    patch2_d = nc.declare_dram_parameter("patch2", [SH, 1], fp, isOutput=False)
    bias1_d = nc.declare_dram_parameter("bias1", [128, 128], fp, isOutput=False)
    out_d = nc.declare_dram_parameter("out", [SH, 128], fp, isOutput=True)

    haug1_loc = nc.dram_tensor("haug1_loc", [SH, 132], fp)
    ad1_loc = nc.dram_tensor("ad1_loc", [SH, 4], fp)
    x2_loc = nc.dram_tensor("x2_loc", [SH, 128], fp)
    haug2_loc = nc.dram_tensor("haug2_loc", [SH, 129], fp)
    ad2_loc = nc.dram_tensor("ad2_loc", [SH, 1], fp)
    haug1_tab = nc.dram_tensor("haug1_tab", [NPAD, 132], fp, addr_space="Shared")
    haug2_tab = nc.dram_tensor("haug2_tab", [NPAD, 129], fp, addr_space="Shared")

    def vap(t, free_dims):
        """View of a tile with explicit free-dim [step, count] pairs."""
        a = t[tuple([slice(None)] * len(t.shape))]
        return bass.AP(tensor=a.tensor, offset=a.offset, ap=[a.ap[0]] + free_dims)

    with tile.TileContext(nc) as tc, ExitStack() as ctx:
        consts = ctx.enter_context(tc.tile_pool(name="consts", bufs=1))
        ident = consts.tile([128, 128], fp)
        make_identity(nc, ident[:])
        # PE observes the identity producer's clock once, so per-tile
        # transposes carry a single (DMA) wait — walrus's LDWEIGHTS struct
        # only tolerates one sync wait.
        warm_ps = ctx.enter_context(tc.tile_pool(name="warm_ps", bufs=2, space="PSUM"))

        def pe_sync():
            # Dummy PE matmul: absorbs cross-engine waits (Pool/SP barrier)
            # so real transposes/matmuls carry a single sync wait each.
            pe_warm = warm_ps.tile([128, 1], fp, space="PSUM", tag="pe_warm")
            nc.tensor.matmul(out=pe_warm[:], lhsT=ident[:], rhs=ident[:, 0:1],
                             start=True, stop=True)

        pe_sync()
        wc1_dma = consts.tile([128, 136], fp)
        nc.sync.dma_start(out=wc1_dma[:], in_=wcat1_d[:, :])
        wc1_sb = consts.tile([128, 136], fp)
        nc.vector.tensor_copy(out=wc1_sb[:], in_=wc1_dma[:])
        wc2_dma = consts.tile([128, 130], fp)
        nc.sync.dma_start(out=wc2_dma[:], in_=wcat2_d[:, :])
        wc2_sb = consts.tile([128, 130], fp)
        nc.vector.tensor_copy(out=wc2_sb[:], in_=wc2_dma[:])
        b1_sb = consts.tile([128, 128], fp)
        nc.sync.dma_start(out=b1_sb[:], in_=bias1_d[:, :])
        offs_sb = consts.tile([128, S], mybir.dt.int32)
        nc.sync.dma_start(out=offs_sb[:], in_=offsets_d[:, :])

        mm_x = ctx.enter_context(tc.tile_pool(name="mm_x", bufs=3))
        mm_ps = ctx.enter_context(tc.tile_pool(name="mm_ps", bufs=2, space="PSUM"))
        mm_st = ctx.enter_context(tc.tile_pool(name="mm_st", bufs=3))

        def matmul_phase(src_dram, wc_sb, ncols, patch_dram, pw, haug_dram, hw,
                         ad_dram, elu_in: bool):
            """haug rows = [h | alpha_src(+patch)] ; alpha_dst -> ad_dram."""
            for t in range(NB):
                r0 = t * 128
                x_t = mm_x.tile([128, 128], fp, tag="x_t")
                nc.sync.dma_start(out=x_t[:], in_=src_dram[r0:r0 + 128, :])
                if elu_in:
                    # x2 = elu(raw + b1):  max(z,0) + exp(min(z,0)) - 1
                    z = mm_x.tile([128, 128], fp, tag="z")
                    nc.vector.tensor_tensor(out=z[:], in0=x_t[:], in1=b1_sb[:],
                                            op=mybir.AluOpType.add)
                    nc.vector.tensor_scalar_max(x_t[:], z[:], 0.0)
                    nc.vector.tensor_scalar_min(z[:], z[:], 0.0)
                    nc.scalar.activation(z[:], z[:], mybir.ActivationFunctionType.Exp)
                    nc.vector.tensor_tensor(out=x_t[:], in0=x_t[:], in1=z[:],
                                            op=mybir.AluOpType.add)
                    nc.vector.tensor_scalar_add(x_t[:], x_t[:], -1.0)
                xt_ps = mm_ps.tile([128, 128], fp, space="PSUM", tag="xt_ps")
                nc.tensor.transpose(out=xt_ps[:], in_=x_t[:], identity=ident[:])
                xt_sb = mm_x.tile([128, 128], fp, tag="xt_sb")
                nc.vector.tensor_copy(out=xt_sb[:], in_=xt_ps[:])
                o_ps = mm_ps.tile([128, ncols], fp, space="PSUM", tag="o_ps")
                nc.tensor.matmul(out=o_ps[:], lhsT=xt_sb[:], rhs=wc_sb[:, :ncols],
                                 start=True, stop=True)
                st = mm_st.tile([128, ncols], fp, tag="st")
                nc.vector.tensor_copy(out=st[:], in_=o_ps[:])
                pt = mm_st.tile([128, pw], fp, tag="pt")
                nc.sync.dma_start(out=pt[:], in_=patch_dram[r0:r0 + 128, :])
                nc.vector.tensor_tensor(out=st[:, 128:128 + pw],
                                        in0=st[:, 128:128 + pw], in1=pt[:],
                                        op=mybir.AluOpType.add)
                nc.sync.dma_start(out=haug_dram[r0:r0 + 128, 0:hw], in_=st[:, 0:hw])
                nc.sync.dma_start(out=ad_dram[r0:r0 + 128, :],
                                  in_=st[:, 128 + pw:128 + 2 * pw])

        eg_g = ctx.enter_context(tc.tile_pool(name="eg_g", bufs=4))
        eg_w = ctx.enter_context(tc.tile_pool(name="eg_w", bufs=3))
        eg_m = ctx.enter_context(tc.tile_pool(name="eg_m", bufs=2))
        eg_s = ctx.enter_context(tc.tile_pool(name="eg_s", bufs=3))
        eg_o = ctx.enter_context(tc.tile_pool(name="eg_o", bufs=3))

        def edge_phase(haug_tab, ad_dram, H, out_dram, W):
            Cc = 128 // H
            for b in range(NB):
                d = int(Dm[b])
                o = int(offs[b])
                r0 = b * 128
                ad_t = eg_s.tile([128, H], fp, tag="ad")
                nc.sync.dma_start(out=ad_t[:], in_=ad_dram[r0:r0 + 128, :])
                G = eg_g.tile([128, d, W], fp, tag="G")
                # HW indirect DMA only honors the one-offset-per-partition
                # form (one descriptor per partition): gather one slot-column
                # (128 rows x W) per call.
                for s in range(d):
                    nc.gpsimd.indirect_dma_start(
                        out=G[:, s, :], out_offset=None, in_=haug_tab[:, :],
                        in_offset=bass.IndirectOffsetOnAxis(
                            ap=offs_sb[:, o + s:o + s + 1], axis=0))
                # logit = alpha_src[src] + alpha_dst[dst]
                logit = eg_w.tile([128, d, H], fp, tag="logit")
                if H == 1:
                    nc.vector.tensor_scalar(
                        out=logit[:, :, 0:1], in0=G[:, :, 128:129],
                        scalar1=ad_t[:, 0:1], scalar2=None,
                        op0=mybir.AluOpType.add)
                else:
                    nc.vector.tensor_tensor(
                        out=logit[:, :, :], in0=G[:, :, 128:128 + H],
                        in1=vap(ad_t, [[0, d], [1, H]]), op=mybir.AluOpType.add)
                # w = exp(leaky_relu(logit)) = max(exp(logit), exp(0.2 logit))
                e1 = eg_w.tile([128, d, H], fp, tag="e1")
                fl = lambda t: t[:].rearrange("p k h -> p (k h)")
                nc.scalar.activation(fl(e1), fl(logit), mybir.ActivationFunctionType.Exp)
                wt = eg_w.tile([128, d, H], fp, tag="wt")
                nc.scalar.activation(fl(wt), fl(logit), mybir.ActivationFunctionType.Exp,
                                     scale=0.2)
                nc.vector.tensor_tensor(out=wt[:, :, :], in0=wt[:, :, :],
                                        in1=e1[:, :, :], op=mybir.AluOpType.max)
                # den/recip
                den = eg_s.tile([128, H], fp, tag="den")
                nc.vector.tensor_reduce(out=den[:, :], in_=vap(wt, [[1, H], [H, d]]),
                                        axis=mybir.AxisListType.X, op=mybir.AluOpType.add)
                nc.vector.tensor_scalar_add(den[:, :], den[:, :], 1e-30)
                rec = eg_s.tile([128, H], fp, tag="rec")
                nc.vector.reciprocal(rec[:, :], den[:, :])
                # msg = w (bcast over Cc) * h[src]
                msg = eg_m.tile([128, d, 128], fp, tag="msg")
                nc.vector.tensor_tensor(
                    out=vap(msg, [[128, d], [Cc, H], [1, Cc]]),
                    in0=vap(G, [[W, d], [Cc, H], [1, Cc]]),
                    in1=vap(wt, [[H, d], [1, H], [0, Cc]]),
                    op=mybir.AluOpType.mult)
                num = eg_o.tile([128, 128], fp, tag="num")
                nc.vector.tensor_reduce(out=num[:, :], in_=vap(msg, [[1, 128], [128, d]]),
                                        axis=mybir.AxisListType.X, op=mybir.AluOpType.add)
                outt = eg_o.tile([128, 128], fp, tag="outt")
                if H == 1:
                    nc.vector.tensor_scalar_mul(outt[:, :], num[:, :], rec[:, 0:1])
                else:
                    nc.vector.tensor_tensor(
                        out=vap(outt, [[Cc, H], [1, Cc]]),
                        in0=vap(num, [[Cc, H], [1, Cc]]),
                        in1=vap(rec, [[1, H], [0, Cc]]),
                        op=mybir.AluOpType.mult)
                nc.sync.dma_start(out=out_dram[r0:r0 + 128, :], in_=outt[:, :])

        # ---------------- layer 1 ----------------
        for _rep in range(reps):
            matmul_phase(x_shard, wc1_sb, 136, patch1_d, 4, haug1_loc, 132, ad1_loc,
                         elu_in=False)
            tc.strict_bb_all_engine_barrier()
            nc.gpsimd.collective_compute(
                "AllGather", mybir.AluOpType.bypass,
                ins=[haug1_loc[:, :]], outs=[haug1_tab[:, :]],
                replica_groups=AG_GROUPS)
            tc.strict_bb_all_engine_barrier()
            if stages >= 2:
                edge_phase(haug1_tab, ad1_loc, cfg.H1, x2_loc, 132)
                tc.strict_bb_all_engine_barrier()
                pe_sync()
            if stages >= 3:
                # ---------------- layer 2 ----------------
                matmul_phase(x2_loc, wc2_sb, 130, patch2_d, 1, haug2_loc, 129, ad2_loc,
                             elu_in=True)
                tc.strict_bb_all_engine_barrier()
                nc.gpsimd.collective_compute(
                    "AllGather", mybir.AluOpType.bypass,
                    ins=[haug2_loc[:, :]], outs=[haug2_tab[:, :]],
                    replica_groups=AG_GROUPS)
                tc.strict_bb_all_engine_barrier()
            if stages >= 4:
                edge_phase(haug2_tab, ad2_loc, 1, out_d, 129)

    if split:
        split_multi_waits(nc)
    return nc


# ---------------------------------------------------------------- entry point

def run(cfg, inputs, trace=False):
    from concourse.bass_utils import run_bass_kernel_spmd

    x = np.asarray(inputs["x"], dtype=np.float32)
    edge_index = np.asarray(inputs["edge_index"]).astype(np.int64)
    perms, Dm, offs, S, offsets = host_prep(cfg, edge_index)
    wcat1, wcat2 = make_wcats(
        cfg, np.asarray(inputs["W1"], np.float32), np.asarray(inputs["a_src1"], np.float32),
        np.asarray(inputs["a_dst1"], np.float32), np.asarray(inputs["W2"], np.float32),
        np.asarray(inputs["a_src2"], np.float32), np.asarray(inputs["a_dst2"], np.float32))
    in_maps = make_core_inputs(cfg, x, perms, offsets, wcat1, wcat2,
                               np.asarray(inputs["b1"], np.float32))
    nc = build_bass(cfg, Dm, offs, S)
    res = run_bass_kernel_spmd(nc, in_maps, list(range(cfg.NCORES)), trace=trace)

    out = np.zeros((cfg.N, 128), np.float32)
    for c in range(cfg.NCORES):
        base = c * cfg.SH
        cnt = min(cfg.SH, cfg.N - base)
        perm = perms[c]
        valid = perm < cnt
        shard = res.results[c]["out"]
        out[base + perm[valid]] = shard[valid]
    out += np.asarray(inputs["b2"], np.float32)[None, :]
    return out, res


def kernel(**inputs) -> np.ndarray:
    out, _ = run(FULL, inputs, trace=False)
    return out
